# revision 2
# baseline (speedup 1.0000x reference)
"""Trainium2 Bass kernel: 1-layer transformer block w/ ALiBi bidirectional attention.

Sharding: data-parallel over batch (B=8) across 8 NeuronCores; zero collectives.

Per-core dataflow (S=1024, D=512, H=8, HD=64, FFN=2048), all fp32:
  - Activations kept in natural layout [s, d] for LayerNorm (bn_stats over free
    dim), transposed via PE (identity matmul) to [d, s] where matmuls need it.
  - ALiBi factorization: bias(s,t) = +-slope*(t - s) splits into a per-s term
    (folded into an augmented K=65 row of the q operand) and a per-t term
    (applied as the per-partition ACT bias of the fused exp, since scores are
    computed transposed: [t partitions, s free]).
  - Each head is half-masked (-1e9) => only the triangular half of the S x S
    score tiles is computed. Diagonal 128x128 tiles are masked by elementwise
    multiply with a 0/1 triangle.
  - softmax denominator r[s] comes free as an extra output row of the
    probs@V matmul (ones column appended to V); 1/r applied after transposing
    attn back to natural layout (per-partition scalar multiply).
  - LN scale/bias of all three LNs folded into the following weight matrices
    host-side (exact algebra, no approximation).
"""

import sys

import numpy as np

sys.path.insert(0, "/opt/trn_rl_repo")

import concourse.bass as bass  # noqa: E402
from concourse import bacc  # noqa: E402
import concourse.tile as tile  # noqa: E402
from concourse import mybir  # noqa: E402
from concourse.bass_utils import run_bass_kernel_spmd  # noqa: E402

F32 = mybir.dt.float32
AF = mybir.ActivationFunctionType
OP = mybir.AluOpType

P = 128
B = 8
S = 1024
D = 512
H = 8
HD = 64
FFN = 4 * D
SM = S // P  # 8 sequence chunks
DK = D // P  # 4 feature chunks
FK = FFN // P  # 16 ffn chunks
EPS = 1e-5
N_CORES = 8


def _slopes():
    half = H // 2
    base = 24.0 ** (1.0 / half)
    return (1.0 / base ** np.arange(1, half + 1)).astype(np.float64)


def _fwd(h):
    return h < H // 2


def _slope(h):
    s = _slopes()
    return float(s[h % (H // 2)])


# per (head, j) score-tile geometry for the transposed scores [t=j*128+p, s]
def _s_range(h, j):
    if _fwd(h):  # keep t <= s : s-chunks j..7
        return j * P, S - j * P
    else:  # keep t >= s : s-chunks 0..j
        return 0, (j + 1) * P


def _eoff(h, j):
    off = 0
    for jj in range(j):
        off += _s_range(h, jj)[1]
    return off


def _ewidth(h):
    return _eoff(h, SM - 1) + _s_range(h, SM - 1)[1]  # = 4608


def build_nc(gelu_mode="gelu"):
    nc = bacc.Bacc("TRN2", target_bir_lowering=False, debug=False)

    def din(name, shape):
        return nc.dram_tensor(name, list(shape), F32, kind="ExternalInput").ap()

    d = {}
    d["x"] = din("x", (S, D))
    d["w_in"] = din("w_in", (D, D))
    d["b_in"] = din("b_in", (D,))
    d["wq"] = din("wq", (D, D))
    d["wk"] = din("wk", (D, D))
    d["wv"] = din("wv", (D, D))
    d["wo"] = din("wo", (D, D))
    d["bo"] = din("bo", (D,))
    d["w1"] = din("w1", (D, FFN))
    d["w2"] = din("w2", (FFN, D))
    d["b2"] = din("b2", (D,))
    d["w_out"] = din("w_out", (D, D))
    d["b_out"] = din("b_out", (D,))
    d["bqc"] = din("bqc", (P, DK))
    d["bkc"] = din("bkc", (P, DK))
    d["b1c"] = din("b1c", (P, FK))
    d["bv"] = din("bv", (D,))
    d["qrow"] = din("qrow", (H, S))
    d["tb"] = din("tb", (P, H * SM))
    d["maskf"] = din("maskf", (P, P))
    d["maskb"] = din("maskb", (P, P))
    d["ident"] = din("ident", (P, P))
    d["out"] = nc.dram_tensor("out", [S, D], F32, kind="ExternalOutput").ap()

    with tile.TileContext(nc) as tc:
        _emit(nc, tc, d, gelu_mode)
    nc.compile()
    return nc


def _emit(nc, tc, d, gelu_mode):
    pool = tc.alloc_tile_pool

    pc = pool(name="consts", bufs=1)
    ph = pool(name="resid", bufs=2)  # tag "h": xn, h1, h2, h3 rotate
    phT = pool(name="transposed", bufs=2)  # tag "hT": xT,hn1T,attnT2,hn2T,hn3T
    psm = pool(name="smalls", bufs=4)
    phn = pool(name="hn_nat", bufs=2)
    pstage = pool(name="stage", bufs=2)
    pg = pool(name="gelu", bufs=3)
    posb = pool(name="outsb", bufs=3)
    pasb = pool(name="attnTsb", bufs=4)

    ps_mm = pool(name="ps_mm", bufs=2, space="PSUM")
    ps_acc = pool(name="ps_acc", bufs=4, space="PSUM")
    ps_tr = pool(name="ps_tr", bufs=2, space="PSUM")

    # wo prefetches early; small enough to keep resident until the end
    pwo = pool(name="wo_pool", bufs=1)
    wo_sb = pwo.tile([P, DK, D], F32, tag="wo")
    nc.sync.dma_start(out=wo_sb, in_=d["wo"].rearrange("(c p) n -> p c n", p=P))

    # ---- constants ----
    ident = pc.tile([P, P], F32, tag="ident")
    nc.sync.dma_start(out=ident, in_=d["ident"])
    maskf = pc.tile([P, P], F32, tag="maskf")
    nc.sync.dma_start(out=maskf, in_=d["maskf"])
    maskb = pc.tile([P, P], F32, tag="maskb")
    nc.sync.dma_start(out=maskb, in_=d["maskb"])
    tb = pc.tile([P, H * SM], F32, tag="tb")
    nc.sync.dma_start(out=tb, in_=d["tb"])
    bqc = pc.tile([P, DK], F32, tag="bqc")
    nc.sync.dma_start(out=bqc, in_=d["bqc"])
    bkc = pc.tile([P, DK], F32, tag="bkc")
    nc.sync.dma_start(out=bkc, in_=d["bkc"])
    b1c = pc.tile([P, FK], F32, tag="b1c")
    nc.sync.dma_start(out=b1c, in_=d["b1c"])
    b1cs = pc.tile([P, FK], F32, tag="b1cs")
    nc.any.tensor_scalar(b1cs, b1c, scalar1=1.702, scalar2=None, op0=OP.mult)

    def bcast(name):
        t = pc.tile([P, D], F32, tag=name + "B")
        nc.gpsimd.dma_start(out=t, in_=d[name].partition_broadcast(P))
        return t

    epsc = pc.tile([P, 1], F32, tag="epsc")
    nc.any.memset(epsc, EPS)

    binB = bcast("b_in")
    bvB = bcast("bv")
    boB = bcast("bo")
    b2B = bcast("b2")
    boutB = bcast("b_out")

    # ---- phase A: load x + first-stage weights ----
    # pool stack order (LIFO release): pqk, pva below pwqkv; pan/pexp pushed
    # after pwqkv pops; pwbig pushed after all attention pools pop.
    pqk = pool(name="qkheads", bufs=2)
    pva = pool(name="vaug", bufs=1)
    pexp = pool(name="expT", bufs=1)
    pwqkv = pool(name="wqkv", bufs=1)
    win_sb = pwqkv.tile([P, DK, D], F32, tag="w_in")
    nc.sync.dma_start(out=win_sb, in_=d["w_in"].rearrange("(c p) n -> p c n", p=P))
    wq_sb = pwqkv.tile([P, DK, D], F32, tag="wq")
    nc.sync.dma_start(out=wq_sb, in_=d["wq"].rearrange("(c p) n -> p c n", p=P))
    wk_sb = pwqkv.tile([P, DK, D], F32, tag="wk")
    nc.sync.dma_start(out=wk_sb, in_=d["wk"].rearrange("(c p) n -> p c n", p=P))
    wv_sb = pwqkv.tile([P, DK, D], F32, tag="wv")
    nc.sync.dma_start(out=wv_sb, in_=d["wv"].rearrange("(c p) n -> p c n", p=P))

    xn = ph.tile([P, SM, D], F32, tag="h")
    nc.sync.dma_start(out=xn, in_=d["x"].rearrange("(c p) n -> p c n", p=P))

    def transpose_to(dst, src):
        # src [128,128] SBUF -> dst [128,128] (SBUF dest via PSUM bounce)
        t = ps_tr.tile([P, P], F32, tag="tr")
        nc.tensor.transpose(t, src, ident)
        nc.any.tensor_copy(dst, t)

    # xT = x transposed [d, s]
    xT = phT.tile([P, DK, S], F32, tag="hT")
    for m in range(SM):
        for dk in range(DK):
            transpose_to(
                xT[:, dk, m * P : (m + 1) * P], xn[:, m, dk * P : (dk + 1) * P]
            )

    # h1 = x @ w_in + b_in    (natural)
    h1 = ph.tile([P, SM, D], F32, tag="h")
    for m in range(SM):
        ps = ps_mm.tile([P, D], F32, tag="mm")
        for dk in range(DK):
            nc.tensor.matmul(
                ps,
                xT[:, dk, m * P : (m + 1) * P],
                win_sb[:, dk, :],
                start=(dk == 0),
                stop=(dk == DK - 1),
            )
        nc.any.tensor_tensor(out=h1[:, m, :], in0=ps, in1=binB, op=OP.add)

    def ln_chunk(src):
        # plain LayerNorm (no scale/bias; those are folded into weights)
        stats = psm.tile([P, 6], F32, tag="st")
        nc.vector.bn_stats(stats, src)
        mv = psm.tile([P, 2], F32, tag="mv")
        nc.vector.bn_aggr(mv, stats)
        sq = psm.tile([P, 1], F32, tag="sq")
        nc.scalar.activation(sq, mv[:, 1:2], AF.Sqrt, bias=epsc)
        rstd = psm.tile([P, 1], F32, tag="rstd")
        nc.vector.reciprocal(rstd, sq)
        negmr = psm.tile([P, 1], F32, tag="negmr")
        nc.vector.tensor_scalar(
            negmr, mv[:, 0:1], scalar1=rstd, scalar2=-1.0, op0=OP.mult, op1=OP.mult
        )
        hn = phn.tile([P, D], F32, tag="hn")
        nc.any.tensor_scalar(
            hn, src, scalar1=rstd, scalar2=negmr, op0=OP.mult, op1=OP.add
        )
        return hn

    # hn1T = LN1(h1) transposed [d, s]
    hn1T = phT.tile([P, DK, S], F32, tag="hT")
    for m in range(SM):
        hn = ln_chunk(h1[:, m, :])
        for dk in range(DK):
            transpose_to(
                hn1T[:, dk, m * P : (m + 1) * P], hn[:, dk * P : (dk + 1) * P]
            )

    # ---- v projection, natural layout, augmented with ones column ----
    v_aug = pva.tile([P, SM, H, 65], F32, tag="vaug")
    for t in range(SM):
        psv = ps_mm.tile([P, D], F32, tag="mm", name="psv")
        for dk in range(DK):
            nc.tensor.matmul(
                psv,
                hn1T[:, dk, t * P : (t + 1) * P],
                wv_sb[:, dk, :],
                start=(dk == 0),
                stop=(dk == DK - 1),
            )
        for h in range(H):
            nc.any.tensor_tensor(
                out=v_aug[:, t, h, 0:64],
                in0=psv[:, h * HD : (h + 1) * HD],
                in1=bvB[:, h * HD : (h + 1) * HD],
                op=OP.add,
            )
        nc.any.memset(v_aug[:, t, :, 64:65], 1.0)

    # ---- attention: interleave per-dout q/k projection with its 2 heads ----
    attn_nat = ph.tile([P, SM, D], F32, tag="h", name="attn_nat")
    for dout in range(DK):
        heads = (2 * dout, 2 * dout + 1)
        qTa = {}
        kTa = {}
        for h in heads:
            qTa[h] = pqk.tile([65, S], F32, tag="qTa", name=f"qTa{h}")
            nc.sync.dma_start(out=qTa[h][64:65, :], in_=d["qrow"][h : h + 1, :])
            kTa[h] = pqk.tile([65, S], F32, tag="kTa", name=f"kTa{h}")
            nc.any.memset(kTa[h][64:65, :], 1.0)
        for w_sb, bc, dst in ((wq_sb, bqc, qTa), (wk_sb, bkc, kTa)):
            for half in range(2):
                psq = ps_mm.tile([P, D], F32, tag="mm", name="psq")
                for dk in range(DK):
                    nc.tensor.matmul(
                        psq,
                        w_sb[:, dk, dout * P : (dout + 1) * P],
                        hn1T[:, dk, half * 512 : (half + 1) * 512],
                        start=(dk == 0),
                        stop=(dk == DK - 1),
                    )
                stg = pstage.tile([P, 512], F32, tag="stg")
                nc.any.tensor_scalar(
                    stg, psq, scalar1=bc[:, dout : dout + 1], scalar2=None,
                    op0=OP.add,
                )
                # split the two heads to partitions 0..63 via DMA
                nc.sync.dma_start(
                    out=dst[heads[0]][0:64, half * 512 : (half + 1) * 512],
                    in_=stg[0:64, :],
                )
                nc.sync.dma_start(
                    out=dst[heads[1]][0:64, half * 512 : (half + 1) * 512],
                    in_=stg[64:128, :],
                )
        for h in heads:
            expT = pexp.tile([P, _ewidth(h)], F32, tag="expT", name=f"expT{h}")
            for j in range(SM):
                s0, w = _s_range(h, j)
                eo = _eoff(h, j)
                off = 0
                while off < w:
                    pw = min(512, w - off)
                    pss = ps_mm.tile([P, pw], F32, tag="mm", name="pss")
                    nc.tensor.matmul(
                        pss,
                        kTa[h][:, j * P : (j + 1) * P],
                        qTa[h][:, s0 + off : s0 + off + pw],
                        start=True,
                        stop=True,
                    )
                    nc.scalar.activation(
                        expT[:, eo + off : eo + off + pw],
                        pss,
                        AF.Exp,
                        bias=tb[:, h * SM + j : h * SM + j + 1],
                        scale=0.125,
                    )
                    off += pw
                # mask the diagonal 128x128 block (keep t<=s fwd / t>=s bwd)
                dg = eo if _fwd(h) else eo + j * P
                msk = maskf if _fwd(h) else maskb
                nc.any.tensor_tensor(
                    out=expT[:, dg : dg + P],
                    in0=expT[:, dg : dg + P],
                    in1=msk,
                    op=OP.mult,
                )
            # probs @ V (unnormalized) + denominator row, per s-chunk
            for m in range(SM):
                js = list(range(0, m + 1)) if _fwd(h) else list(range(m, SM))
                pv = ps_acc.tile([65, P], F32, tag="acc", name="pvps")
                for i, j in enumerate(js):
                    s0, _w = _s_range(h, j)
                    col = _eoff(h, j) + (m * P - s0)
                    nc.tensor.matmul(
                        pv,
                        v_aug[:, j, h, :],
                        expT[:, col : col + P],
                        start=(i == 0),
                        stop=(i == len(js) - 1),
                    )
                asb = pasb.tile([65, P], F32, tag="asb")
                nc.any.tensor_copy(asb, pv)
                trp = ps_tr.tile([P, 65], F32, tag="tr", name="atr")
                nc.tensor.transpose(trp, asb, ident[0:65, 0:65])
                rinv = psm.tile([P, 1], F32, tag="rinv")
                nc.vector.reciprocal(rinv, trp[:, 64:65])
                nc.any.tensor_scalar(
                    attn_nat[:, m, h * HD : (h + 1) * HD],
                    trp[:, 0:64],
                    scalar1=rinv,
                    scalar2=None,
                    op0=OP.mult,
                )

    pwqkv.release()
    pexp.release()

    # attn transposed for the output projection
    attnT2 = phT.tile([P, DK, S], F32, tag="hT")
    for m in range(SM):
        for dk in range(DK):
            transpose_to(
                attnT2[:, dk, m * P : (m + 1) * P],
                attn_nat[:, m, dk * P : (dk + 1) * P],
            )

    # h2 = h1 + attn @ wo + bo
    h2 = ph.tile([P, SM, D], F32, tag="h")
    for m in range(SM):
        ps = ps_mm.tile([P, D], F32, tag="mm", name="pswo")
        for dk in range(DK):
            nc.tensor.matmul(
                ps,
                attnT2[:, dk, m * P : (m + 1) * P],
                wo_sb[:, dk, :],
                start=(dk == 0),
                stop=(dk == DK - 1),
            )
        nc.any.tensor_tensor(out=h2[:, m, :], in0=ps, in1=h1[:, m, :], op=OP.add)
        nc.any.tensor_tensor(out=h2[:, m, :], in0=h2[:, m, :], in1=boB, op=OP.add)

    pva.release()
    pqk.release()

    # big tail weights stream in as soon as attention SBUF frees up
    pwbig = pool(name="wbig", bufs=1)
    w1_sb = pwbig.tile([P, DK, FFN], F32, tag="w1")
    nc.sync.dma_start(out=w1_sb, in_=d["w1"].rearrange("(c p) n -> p c n", p=P))
    w2_sb = pwbig.tile([P, FK, D], F32, tag="w2")
    nc.sync.dma_start(out=w2_sb, in_=d["w2"].rearrange("(c p) n -> p c n", p=P))
    wout_sb = pwbig.tile([P, DK, D], F32, tag="w_out")
    nc.sync.dma_start(out=wout_sb, in_=d["w_out"].rearrange("(c p) n -> p c n", p=P))

    # hn2T = LN2(h2) transposed
    hn2T = phT.tile([P, DK, S], F32, tag="hT")
    for m in range(SM):
        hn = ln_chunk(h2[:, m, :])
        for dk in range(DK):
            transpose_to(
                hn2T[:, dk, m * P : (m + 1) * P], hn[:, dk * P : (dk + 1) * P]
            )

    # ---- FFN: h3 = h2 + gelu(hn2 @ w1 + b1) @ w2 + b2 ----
    h3 = ph.tile([P, SM, D], F32, tag="h")
    for half in range(2):
        accs = []
        for mm in range(4):
            accs.append(ps_acc.tile([P, D], F32, tag="acc", name=f"ff2ps{mm}"))
        for kc in range(FK):
            ps1 = ps_mm.tile([P, 512], F32, tag="mm", name="ff1ps")
            for dk in range(DK):
                nc.tensor.matmul(
                    ps1,
                    w1_sb[:, dk, kc * P : (kc + 1) * P],
                    hn2T[:, dk, half * 512 : (half + 1) * 512],
                    start=(dk == 0),
                    stop=(dk == DK - 1),
                )
            gt = pg.tile([P, 512], F32, tag="gt")
            if gelu_mode == "gelu":
                nc.scalar.activation(gt, ps1, AF.Gelu, bias=b1c[:, kc : kc + 1])
            else:  # CoreSim lacks Gelu: x*sigmoid(1.702x) stand-in
                sg = pg.tile([P, 512], F32, tag="sg")
                nc.scalar.activation(
                    sg, ps1, AF.Sigmoid, bias=b1cs[:, kc : kc + 1], scale=1.702
                )
                xb = pg.tile([P, 512], F32, tag="xb")
                nc.any.tensor_scalar(
                    xb, ps1, scalar1=b1c[:, kc : kc + 1], scalar2=None, op0=OP.add
                )
                nc.any.tensor_tensor(out=gt, in0=sg, in1=xb, op=OP.mult)
            for mm in range(4):
                nc.tensor.matmul(
                    accs[mm],
                    gt[:, mm * P : (mm + 1) * P],
                    w2_sb[:, kc, :],
                    start=(kc == 0),
                    stop=(kc == FK - 1),
                )
        for mm in range(4):
            m = half * 4 + mm
            nc.any.tensor_tensor(
                out=h3[:, m, :], in0=accs[mm], in1=h2[:, m, :], op=OP.add
            )
            nc.any.tensor_tensor(out=h3[:, m, :], in0=h3[:, m, :], in1=b2B, op=OP.add)

    # ---- final LN + output projection ----
    hn3T = phT.tile([P, DK, S], F32, tag="hT")
    for m in range(SM):
        hn = ln_chunk(h3[:, m, :])
        for dk in range(DK):
            transpose_to(
                hn3T[:, dk, m * P : (m + 1) * P], hn[:, dk * P : (dk + 1) * P]
            )

    out_view = d["out"].rearrange("(c p) n -> p c n", p=P)
    for m in range(SM):
        ps = ps_mm.tile([P, D], F32, tag="mm", name="psout")
        for dk in range(DK):
            nc.tensor.matmul(
                ps,
                hn3T[:, dk, m * P : (m + 1) * P],
                wout_sb[:, dk, :],
                start=(dk == 0),
                stop=(dk == DK - 1),
            )
        osb = posb.tile([P, D], F32, tag="osb")
        nc.any.tensor_tensor(out=osb, in0=ps, in1=boutB, op=OP.add)
        nc.sync.dma_start(out=out_view[:, m, :], in_=osb)

    for p_ in (pwbig, pwo, ps_tr, ps_acc, ps_mm, pasb, posb, pg, pstage,
               phn, psm, phT, ph, pc):
        p_.release()


def host_prep(inputs):
    """Fold LN affine params into weights; build ALiBi helper tensors."""
    f = lambda k: np.asarray(inputs[k], dtype=np.float64)
    ln1_s, ln1_b = f("ln1_s"), f("ln1_b")
    ln2_s, ln2_b = f("ln2_s"), f("ln2_b")
    lnf_s, lnf_b = f("lnf_s"), f("lnf_b")
    wq, bq = f("wq"), f("bq")
    wk, bk = f("wk"), f("bk")
    wv, bv = f("wv"), f("bv")
    w1, b1 = f("w1"), f("b1")
    w_out, b_out = f("w_out"), f("b_out")

    wq_f = (ln1_s[:, None] * wq).astype(np.float32)
    bq_f = (bq + ln1_b @ wq).astype(np.float32)
    wk_f = (ln1_s[:, None] * wk).astype(np.float32)
    bk_f = (bk + ln1_b @ wk).astype(np.float32)
    wv_f = (ln1_s[:, None] * wv).astype(np.float32)
    bv_f = (bv + ln1_b @ wv).astype(np.float32)
    w1_f = (ln2_s[:, None] * w1).astype(np.float32)
    b1_f = (b1 + ln2_b @ w1).astype(np.float32)
    wout_f = (lnf_s[:, None] * w_out).astype(np.float32)
    bout_f = (b_out + lnf_b @ w_out).astype(np.float32)

    sl = _slopes()
    qrow = np.zeros((H, S), np.float32)
    tb = np.zeros((P, H * SM), np.float32)
    s_idx = np.arange(S, dtype=np.float64)
    p_idx = np.arange(P, dtype=np.float64)
    for h in range(H):
        sgn = -1.0 if h < H // 2 else 1.0  # sign of the per-s row term
        qrow[h] = (sgn * 8.0 * sl[h % 4] * s_idx).astype(np.float32)
        for j in range(SM):
            tb[:, h * SM + j] = (-sgn * sl[h % 4] * (j * P + p_idx)).astype(
                np.float32
            )
    maskf = np.triu(np.ones((P, P), np.float32))  # keep t <= s (p <= c)
    maskb = np.tril(np.ones((P, P), np.float32))  # keep t >= s (p >= c)

    common = {
        "w_in": np.asarray(inputs["w_in"], np.float32),
        "b_in": np.asarray(inputs["b_in"], np.float32),
        "wq": wq_f,
        "wk": wk_f,
        "wv": wv_f,
        "wo": np.asarray(inputs["wo"], np.float32),
        "bo": np.asarray(inputs["bo"], np.float32),
        "w1": w1_f,
        "w2": np.asarray(inputs["w2"], np.float32),
        "b2": np.asarray(inputs["b2"], np.float32),
        "w_out": wout_f,
        "b_out": bout_f,
        "bqc": np.ascontiguousarray(bq_f.reshape(DK, P).T),
        "bkc": np.ascontiguousarray(bk_f.reshape(DK, P).T),
        "b1c": np.ascontiguousarray(b1_f.reshape(FK, P).T),
        "bv": bv_f,
        "qrow": qrow,
        "tb": tb,
        "maskf": maskf,
        "maskb": maskb,
        "ident": np.eye(P, dtype=np.float32),
    }
    return common


_NC_CACHE = {}


def get_nc(gelu_mode="gelu"):
    if gelu_mode not in _NC_CACHE:
        _NC_CACHE[gelu_mode] = build_nc(gelu_mode)
    return _NC_CACHE[gelu_mode]


def run(inputs, trace=False, tmpdir=None):
    common = host_prep(inputs)
    x = np.asarray(inputs["x"], np.float32)
    in_maps = [dict(common, x=np.ascontiguousarray(x[i])) for i in range(N_CORES)]
    nc = get_nc()
    res = run_bass_kernel_spmd(
        nc, in_maps, core_ids=list(range(N_CORES)), trace=trace, tmpdir=tmpdir
    )
    out = np.stack([res.results[i]["out"] for i in range(N_CORES)])
    return out.astype(np.float32), res


def kernel(**inputs):
    out, _ = run(inputs, trace=False)
    return out



# revision 8
# speedup vs baseline: 2.2170x; 2.2170x over previous
"""Trainium2 Bass kernel: 1-layer transformer block w/ ALiBi bidirectional attention.

Sharding: data-parallel over batch (B=8) across 8 NeuronCores; zero collectives.

v2 (bf16): all matmuls run in bf16 (1 cyc/row on PE vs 4 for fp32, and
LDWEIGHTS gets fast-weight-load). Residual stream / LN stats / softmax
normalization stay fp32. Other changes vs v1:
  - x is transposed + cast to bf16 on HOST; no on-device xT transposes.
  - q/k projections emit per-head [64, S] psum chunks directly into the
    augmented [65, S] qTa/kTa tiles -> no SBUF->SBUF head-split DMAs.
  - probs@V computed s-major: out[s, 65] = expT_chunk.T @ v_aug, so the
    softmax denominator lands in column 64 and normalization is a
    per-partition scalar multiply; no per-head transposes.
  - ALiBi: per-s term rides the augmented q row (bf16 rounding of it is a
    per-s additive exponent error that cancels exactly in softmax);
    per-t term is the fp32 per-partition ACT bias of the fused exp.
  - LN scale/bias folded into following weight matrices host-side.
"""

import sys

import ml_dtypes
import numpy as np

sys.path.insert(0, "/opt/trn_rl_repo")

import concourse.bass as bass  # noqa: E402
from concourse import bacc  # noqa: E402
import concourse.tile as tile  # noqa: E402
from concourse import mybir  # noqa: E402
from concourse.bass_utils import run_bass_kernel_spmd  # noqa: E402

F32 = mybir.dt.float32
BF = mybir.dt.bfloat16
AF = mybir.ActivationFunctionType
OP = mybir.AluOpType

P = 128
B = 8
S = 1024
D = 512
H = 8
HD = 64
FFN = 4 * D
SM = S // P  # 8 sequence chunks
DK = D // P  # 4 feature chunks
FK = FFN // P  # 16 ffn chunks
EPS = 1e-5
N_CORES = 8

BF_NP = ml_dtypes.bfloat16


def _slopes():
    half = H // 2
    base = 24.0 ** (1.0 / half)
    return (1.0 / base ** np.arange(1, half + 1)).astype(np.float64)


def _fwd(h):
    return h < H // 2


# per (head, j) score-tile geometry for the transposed scores [t=j*128+p, s]
def _s_range(h, j):
    if _fwd(h):  # keep t <= s : s-chunks j..7
        return j * P, S - j * P
    else:  # keep t >= s : s-chunks 0..j
        return 0, (j + 1) * P


def _eoff(h, j):
    off = 0
    for jj in range(j):
        off += _s_range(h, jj)[1]
    return off


def _ewidth(h):
    return _eoff(h, SM - 1) + _s_range(h, SM - 1)[1]  # = 4608


def build_nc(gelu_mode="gelu"):
    nc = bacc.Bacc("TRN2", target_bir_lowering=False, debug=False)

    def din(name, shape, dt=F32):
        return nc.dram_tensor(name, list(shape), dt, kind="ExternalInput").ap()

    d = {}
    d["xT"] = din("xT", (D, S), BF)
    d["w_in"] = din("w_in", (D, D), BF)
    d["b_in"] = din("b_in", (D,))
    d["wq"] = din("wq", (D, D), BF)
    d["wk"] = din("wk", (D, D), BF)
    d["wv"] = din("wv", (D, D), BF)
    d["wo"] = din("wo", (D, D), BF)
    d["bo"] = din("bo", (D,))
    d["w1"] = din("w1", (D, FFN), BF)
    d["w2"] = din("w2", (FFN, D), BF)
    d["b2"] = din("b2", (D,))
    d["w_out"] = din("w_out", (D, D), BF)
    d["b_out"] = din("b_out", (D,))
    d["bqc"] = din("bqc", (HD, H))
    d["b1c"] = din("b1c", (P, FK))
    d["bv"] = din("bv", (D,))
    d["qrow"] = din("qrow", (H, S), BF)
    d["tb"] = din("tb", (P, H * SM))
    d["maskf"] = din("maskf", (P, P), BF)
    d["maskb"] = din("maskb", (P, P), BF)
    d["ident"] = din("ident", (P, P), BF)
    d["out"] = nc.dram_tensor("out", [S, D], F32, kind="ExternalOutput").ap()

    with tile.TileContext(nc) as tc:
        _emit(nc, tc, d, gelu_mode)
    nc.compile()
    return nc


def _emit(nc, tc, d, gelu_mode):
    pool = tc.alloc_tile_pool

    pc = pool(name="consts", bufs=1)
    pw = pool(name="weights", bufs=1)  # all weights resident, bf16
    ph = pool(name="resid", bufs=2)  # tag "h": h1, h2, h3 rotate (fp32)
    phT = pool(name="transposed", bufs=2)  # tag "hT": hn1T,attnT2,hn2T,hn3T
    psm = pool(name="smalls", bufs=4)
    phn = pool(name="hn_nat", bufs=2)
    pg = pool(name="gelu", bufs=3)
    posb = pool(name="outsb", bufs=3)
    pattn = pool(name="attn_nat", bufs=1)
    pva = pool(name="vaug", bufs=1)
    pqk = pool(name="qkheads", bufs=2)
    pexp = pool(name="expT", bufs=2)

    ps_mm = pool(name="ps_mm", bufs=2, space="PSUM")
    ps_acc = pool(name="ps_acc", bufs=4, space="PSUM")
    ps_tr = pool(name="ps_tr", bufs=2, space="PSUM")

    # ---- weights (bf16), staged early; all fit resident ----
    def wload(name, shape, view):
        t = pw.tile(shape, BF, tag=name)
        nc.sync.dma_start(out=t, in_=view)
        return t

    xT_sb = wload("xT", [P, DK, S], d["xT"].rearrange("(c p) n -> p c n", p=P))
    win_sb = wload("w_in", [P, DK, D], d["w_in"].rearrange("(c p) n -> p c n", p=P))
    wq_sb = wload("wq", [P, DK, D], d["wq"].rearrange("(c p) n -> p c n", p=P))
    wk_sb = wload("wk", [P, DK, D], d["wk"].rearrange("(c p) n -> p c n", p=P))
    wv_sb = wload("wv", [P, DK, D], d["wv"].rearrange("(c p) n -> p c n", p=P))
    wo_sb = wload("wo", [P, DK, D], d["wo"].rearrange("(c p) n -> p c n", p=P))
    w1_sb = wload("w1", [P, DK, FFN], d["w1"].rearrange("(c p) n -> p c n", p=P))
    w2_sb = wload("w2", [P, FK, D], d["w2"].rearrange("(c p) n -> p c n", p=P))
    wout_sb = wload("w_out", [P, DK, D], d["w_out"].rearrange("(c p) n -> p c n", p=P))

    # ---- constants ----
    identB = pc.tile([P, P], BF, tag="ident")
    nc.sync.dma_start(out=identB, in_=d["ident"])
    maskf = pc.tile([P, P], BF, tag="maskf")
    nc.sync.dma_start(out=maskf, in_=d["maskf"])
    maskb = pc.tile([P, P], BF, tag="maskb")
    nc.sync.dma_start(out=maskb, in_=d["maskb"])
    tb = pc.tile([P, H * SM], F32, tag="tb")
    nc.sync.dma_start(out=tb, in_=d["tb"])
    bqc = pc.tile([HD, H], F32, tag="bqc")
    nc.sync.dma_start(out=bqc, in_=d["bqc"])
    b1c = pc.tile([P, FK], F32, tag="b1c")
    nc.sync.dma_start(out=b1c, in_=d["b1c"])
    b1cs = pc.tile([P, FK], F32, tag="b1cs")
    nc.any.tensor_scalar(b1cs, b1c, scalar1=1.702, scalar2=None, op0=OP.mult)

    def bcast(name, shape=None):
        t = pc.tile(shape or [P, D], F32, tag=name + "B")
        nc.gpsimd.dma_start(out=t, in_=d[name].partition_broadcast(P))
        return t

    epsc = pc.tile([P, 1], F32, tag="epsc")
    nc.any.memset(epsc, EPS)

    binB = bcast("b_in")
    bvB = bcast("bv", [P, H, HD])
    boB = bcast("bo")
    b2B = bcast("b2")
    boutB = bcast("b_out")

    # ---- h1 = x @ w_in + b_in  (natural fp32, residual base) ----
    h1 = ph.tile([P, SM, D], F32, tag="h")
    for m in range(SM):
        ps = ps_mm.tile([P, D], F32, tag="mm")
        for dk in range(DK):
            nc.tensor.matmul(
                ps,
                xT_sb[:, dk, m * P : (m + 1) * P],
                win_sb[:, dk, :],
                start=(dk == 0),
                stop=(dk == DK - 1),
            )
        nc.vector.tensor_tensor(out=h1[:, m, :], in0=ps, in1=binB, op=OP.add)

    def ln_chunk(src):
        # plain LayerNorm (no scale/bias; those are folded into weights)
        stats = psm.tile([P, 6], F32, tag="st")
        nc.vector.bn_stats(stats, src)
        mv = psm.tile([P, 2], F32, tag="mv")
        nc.vector.bn_aggr(mv, stats)
        sq = psm.tile([P, 1], F32, tag="sq")
        nc.scalar.activation(sq, mv[:, 1:2], AF.Sqrt, bias=epsc)
        rstd = psm.tile([P, 1], F32, tag="rstd")
        nc.vector.reciprocal(rstd, sq)
        negmr = psm.tile([P, 1], F32, tag="negmr")
        nc.vector.tensor_scalar(
            negmr, mv[:, 0:1], scalar1=rstd, scalar2=-1.0, op0=OP.mult, op1=OP.mult
        )
        hn = phn.tile([P, D], BF, tag="hn")
        nc.vector.tensor_scalar(
            hn, src, scalar1=rstd, scalar2=negmr, op0=OP.mult, op1=OP.add
        )
        return hn

    def transpose_to(dst, src, k):
        # src [128,128] bf16 SBUF -> dst [128,128] bf16 (via PSUM bounce)
        # gpsimd cannot read PSUM; alternate DVE / ACT for the bounce copy
        t = ps_tr.tile([P, P], BF, tag="tr")
        nc.tensor.transpose(t, src, identB)
        if k % 2 == 0:
            nc.vector.tensor_copy(dst, t)
        else:
            nc.scalar.copy(dst, t)

    def make_hnT(hsrc, tag_note=""):
        hT = phT.tile([P, DK, S], BF, tag="hT")
        for m in range(SM):
            hn = ln_chunk(hsrc[:, m, :])
            for dk in range(DK):
                transpose_to(
                    hT[:, dk, m * P : (m + 1) * P],
                    hn[:, dk * P : (dk + 1) * P],
                    m * DK + dk,
                )
        return hT

    # hn1T = LN1(h1) transposed [d, s] bf16
    hn1T = make_hnT(h1)

    # ---- v projection -> v_aug [P=t, SM, H, 65] bf16 (ones col for denom) ----
    v_aug = pva.tile([P, SM, H, HD + 1], BF, tag="vaug")
    for t in range(SM):
        psv = ps_mm.tile([P, H, HD], F32, tag="mm", name="psv")
        for dk in range(DK):
            nc.tensor.matmul(
                psv,
                hn1T[:, dk, t * P : (t + 1) * P],
                wv_sb[:, dk, :],
                start=(dk == 0),
                stop=(dk == DK - 1),
            )
        nc.vector.tensor_tensor(out=v_aug[:, t, :, 0:HD], in0=psv, in1=bvB, op=OP.add)
        nc.gpsimd.memset(v_aug[:, t, :, HD : HD + 1], 1.0)

    # ---- attention, head by head ----
    attn_nat = pattn.tile([P, SM, D], BF, tag="attn")
    for h in range(H):
        # q/k projections emitted per-head: psum [64, 512] chunks
        qTa = pqk.tile([HD + 1, S], BF, tag="qTa", name=f"qTa{h}")
        nc.sync.dma_start(out=qTa[HD : HD + 1, :], in_=d["qrow"][h : h + 1, :])
        kTa = pqk.tile([HD + 1, S], BF, tag="kTa", name=f"kTa{h}")
        nc.gpsimd.memset(kTa[HD : HD + 1, :], 1.0)
        for w_sb, dst, is_q in ((wq_sb, qTa, True), (wk_sb, kTa, False)):
            for half in range(2):
                psq = ps_mm.tile([HD, D], F32, tag="mm", name="psq")
                for dk in range(DK):
                    nc.tensor.matmul(
                        psq,
                        w_sb[:, dk, h * HD : (h + 1) * HD],
                        hn1T[:, dk, half * 512 : (half + 1) * 512],
                        start=(dk == 0),
                        stop=(dk == DK - 1),
                    )
                if is_q:
                    nc.scalar.activation(
                        dst[0:HD, half * 512 : (half + 1) * 512],
                        psq,
                        AF.Identity,
                        bias=bqc[:, h : h + 1],
                    )
                else:
                    # k bias dropped: it only shifts scores by a per-s
                    # constant, which softmax normalization cancels exactly
                    nc.vector.tensor_copy(
                        dst[0:HD, half * 512 : (half + 1) * 512], psq
                    )

        # scores -> exp, transposed layout [t partitions, s free]
        expT = pexp.tile([P, _ewidth(h)], BF, tag="expT", name=f"expT{h}")
        for j in range(SM):
            s0, w = _s_range(h, j)
            eo = _eoff(h, j)
            off = 0
            while off < w:
                pw_ = min(512, w - off)
                pss = ps_mm.tile([P, pw_], F32, tag="mm", name="pss")
                nc.tensor.matmul(
                    pss,
                    kTa[:, j * P : (j + 1) * P],
                    qTa[:, s0 + off : s0 + off + pw_],
                    start=True,
                    stop=True,
                )
                nc.scalar.activation(
                    expT[:, eo + off : eo + off + pw_],
                    pss,
                    AF.Exp,
                    bias=tb[:, h * SM + j : h * SM + j + 1],
                    scale=0.125,
                )
                off += pw_
            # mask the diagonal 128x128 block (keep t<=s fwd / t>=s bwd)
            dg = eo if _fwd(h) else eo + j * P
            msk = maskf if _fwd(h) else maskb
            nc.gpsimd.tensor_tensor(
                out=expT[:, dg : dg + P],
                in0=expT[:, dg : dg + P],
                in1=msk,
                op=OP.mult,
            )
        # probs @ V, s-major: out[s, 65]; col 64 = softmax denominator
        for m in range(SM):
            js = list(range(0, m + 1)) if _fwd(h) else list(range(m, SM))
            pv = ps_acc.tile([P, HD + 1], F32, tag="acc", name="pvps")
            for i, j in enumerate(js):
                s0, _w = _s_range(h, j)
                col = _eoff(h, j) + (m * P - s0)
                nc.tensor.matmul(
                    pv,
                    expT[:, col : col + P],
                    v_aug[:, j, h, :],
                    start=(i == 0),
                    stop=(i == len(js) - 1),
                )
            rinv = psm.tile([P, 1], F32, tag="rinv")
            nc.vector.reciprocal(rinv, pv[:, HD : HD + 1])
            if h % 2 == 0:
                nc.vector.tensor_scalar(
                    attn_nat[:, m, h * HD : (h + 1) * HD],
                    pv[:, 0:HD],
                    scalar1=rinv,
                    scalar2=None,
                    op0=OP.mult,
                )
            else:
                nc.scalar.activation(
                    attn_nat[:, m, h * HD : (h + 1) * HD],
                    pv[:, 0:HD],
                    AF.Copy,
                    scale=rinv,
                )

    # attn transposed for the output projection
    attnT2 = phT.tile([P, DK, S], BF, tag="hT")
    for m in range(SM):
        for dk in range(DK):
            transpose_to(
                attnT2[:, dk, m * P : (m + 1) * P],
                attn_nat[:, m, dk * P : (dk + 1) * P],
                m * DK + dk,
            )

    # h2 = h1 + attn @ wo + bo
    h2 = ph.tile([P, SM, D], F32, tag="h")
    for m in range(SM):
        ps = ps_mm.tile([P, D], F32, tag="mm", name="pswo")
        for dk in range(DK):
            nc.tensor.matmul(
                ps,
                attnT2[:, dk, m * P : (m + 1) * P],
                wo_sb[:, dk, :],
                start=(dk == 0),
                stop=(dk == DK - 1),
            )
        nc.vector.tensor_tensor(out=h2[:, m, :], in0=ps, in1=h1[:, m, :], op=OP.add)
        nc.vector.tensor_tensor(out=h2[:, m, :], in0=h2[:, m, :], in1=boB, op=OP.add)

    # hn2T = LN2(h2) transposed bf16
    hn2T = make_hnT(h2)

    # ---- FFN: h3 = h2 + gelu(hn2 @ w1 + b1) @ w2 + b2 ----
    h3 = ph.tile([P, SM, D], F32, tag="h")
    for half in range(2):
        accs = []
        for mm in range(4):
            accs.append(ps_acc.tile([P, D], F32, tag="acc", name=f"ff2ps{mm}"))
        for kc in range(FK):
            ps1 = ps_mm.tile([P, 512], F32, tag="mm", name="ff1ps")
            for dk in range(DK):
                nc.tensor.matmul(
                    ps1,
                    w1_sb[:, dk, kc * P : (kc + 1) * P],
                    hn2T[:, dk, half * 512 : (half + 1) * 512],
                    start=(dk == 0),
                    stop=(dk == DK - 1),
                )
            gt = pg.tile([P, 512], BF, tag="gt")
            if gelu_mode == "gelu":
                nc.scalar.activation(gt, ps1, AF.Gelu, bias=b1c[:, kc : kc + 1])
            else:  # CoreSim lacks Gelu: x*sigmoid(1.702x) stand-in
                sg = pg.tile([P, 512], F32, tag="sg")
                nc.scalar.activation(
                    sg, ps1, AF.Sigmoid, bias=b1cs[:, kc : kc + 1], scale=1.702
                )
                xb = pg.tile([P, 512], F32, tag="xb")
                nc.any.tensor_scalar(
                    xb, ps1, scalar1=b1c[:, kc : kc + 1], scalar2=None, op0=OP.add
                )
                nc.any.tensor_tensor(out=gt, in0=sg, in1=xb, op=OP.mult)
            for mm in range(4):
                nc.tensor.matmul(
                    accs[mm],
                    gt[:, mm * P : (mm + 1) * P],
                    w2_sb[:, kc, :],
                    start=(kc == 0),
                    stop=(kc == FK - 1),
                )
        for mm in range(4):
            m = half * 4 + mm
            nc.vector.tensor_tensor(
                out=h3[:, m, :], in0=accs[mm], in1=h2[:, m, :], op=OP.add
            )
            nc.vector.tensor_tensor(
                out=h3[:, m, :], in0=h3[:, m, :], in1=b2B, op=OP.add
            )

    # ---- final LN + output projection ----
    hn3T = make_hnT(h3)

    out_view = d["out"].rearrange("(c p) n -> p c n", p=P)
    for m in range(SM):
        ps = ps_mm.tile([P, D], F32, tag="mm", name="psout")
        for dk in range(DK):
            nc.tensor.matmul(
                ps,
                hn3T[:, dk, m * P : (m + 1) * P],
                wout_sb[:, dk, :],
                start=(dk == 0),
                stop=(dk == DK - 1),
            )
        osb = posb.tile([P, D], F32, tag="osb")
        nc.vector.tensor_tensor(out=osb, in0=ps, in1=boutB, op=OP.add)
        nc.sync.dma_start(out=out_view[:, m, :], in_=osb)

    for p_ in (ps_tr, ps_acc, ps_mm, pexp, pqk, pva, pattn, posb, pg, phn,
               psm, phT, ph, pw, pc):
        p_.release()


def host_prep(inputs):
    """Fold LN affine params into weights; build ALiBi helper tensors."""
    f = lambda k: np.asarray(inputs[k], dtype=np.float64)
    ln1_s, ln1_b = f("ln1_s"), f("ln1_b")
    ln2_s, ln2_b = f("ln2_s"), f("ln2_b")
    lnf_s, lnf_b = f("lnf_s"), f("lnf_b")
    wq, bq = f("wq"), f("bq")
    wk = f("wk")
    wv, bv = f("wv"), f("bv")
    w1, b1 = f("w1"), f("b1")
    w_out, b_out = f("w_out"), f("b_out")

    wq_f = ln1_s[:, None] * wq
    bq_f = (bq + ln1_b @ wq).astype(np.float32)
    wk_f = ln1_s[:, None] * wk
    wv_f = ln1_s[:, None] * wv
    bv_f = (bv + ln1_b @ wv).astype(np.float32)
    w1_f = ln2_s[:, None] * w1
    b1_f = (b1 + ln2_b @ w1).astype(np.float32)
    wout_f = lnf_s[:, None] * w_out
    bout_f = (b_out + lnf_b @ w_out).astype(np.float32)

    sl = _slopes()
    qrow = np.zeros((H, S), np.float32)
    tb = np.zeros((P, H * SM), np.float32)
    s_idx = np.arange(S, dtype=np.float64)
    p_idx = np.arange(P, dtype=np.float64)
    for h in range(H):
        sgn = -1.0 if h < H // 2 else 1.0  # sign of the per-s row term
        qrow[h] = (sgn * 8.0 * sl[h % 4] * s_idx).astype(np.float32)
        for j in range(SM):
            tb[:, h * SM + j] = (-sgn * sl[h % 4] * (j * P + p_idx)).astype(
                np.float32
            )
    maskf = np.triu(np.ones((P, P), np.float32))  # keep t <= s (p <= c)
    maskb = np.tril(np.ones((P, P), np.float32))  # keep t >= s (p >= c)

    bf = lambda a: np.ascontiguousarray(np.asarray(a, np.float32).astype(BF_NP))
    common = {
        "w_in": bf(inputs["w_in"]),
        "b_in": np.asarray(inputs["b_in"], np.float32),
        "wq": bf(wq_f),
        "wk": bf(wk_f),
        "wv": bf(wv_f),
        "wo": bf(inputs["wo"]),
        "bo": np.asarray(inputs["bo"], np.float32),
        "w1": bf(w1_f),
        "w2": bf(inputs["w2"]),
        "b2": np.asarray(inputs["b2"], np.float32),
        "w_out": bf(wout_f),
        "b_out": bout_f,
        "bqc": np.ascontiguousarray(bq_f.reshape(H, HD).T),
        "b1c": np.ascontiguousarray(b1_f.reshape(FK, P).T),
        "bv": bv_f,
        "qrow": bf(qrow),
        "tb": tb,
        "maskf": bf(maskf),
        "maskb": bf(maskb),
        "ident": bf(np.eye(P, dtype=np.float32)),
    }
    return common


def core_map(common, x, i):
    xT = np.ascontiguousarray(np.asarray(x[i], np.float32).T.astype(BF_NP))
    return dict(common, xT=xT)


_NC_CACHE = {}


def get_nc(gelu_mode="gelu"):
    if gelu_mode not in _NC_CACHE:
        _NC_CACHE[gelu_mode] = build_nc(gelu_mode)
    return _NC_CACHE[gelu_mode]


def run(inputs, trace=False, tmpdir=None):
    common = host_prep(inputs)
    x = np.asarray(inputs["x"], np.float32)
    in_maps = [core_map(common, x, i) for i in range(N_CORES)]
    nc = get_nc()
    res = run_bass_kernel_spmd(
        nc, in_maps, core_ids=list(range(N_CORES)), trace=trace, tmpdir=tmpdir
    )
    out = np.stack([res.results[i]["out"] for i in range(N_CORES)])
    return out.astype(np.float32), res


def kernel(**inputs):
    out, _ = run(inputs, trace=False)
    return out
